# revision 39
# baseline (speedup 1.0000x reference)
"""Deformable-ROI bilinear feature gather (KeypPointBboxNet) on 8 TRN2 cores.

Strategy: feat_map sharded by batch, two images per NeuronCore packed at
int16-index stride H*W (each image pair is held by 2 cores and its points are
split between them in sorted order — near-perfect load balance). The host
precomputes, per sample point, the bilinear cell index and the 4 corner
weights (f32, exactly reproducing the reference's floor/clip/edge rules), so
the device does no coordinate math at all.

Each image is staged in HBM in fp16 "doubled-row" layout g[r] = [fm_row(r) |
fm_row(r+W)], so a single 2KB dma_gather element starting at row r yields all
four bilinear corners [TL, BL, TR, BR] of cell r. Points are sorted by cell
and placed stripe-major (contiguous sorted block per 16-partition idx stripe)
for HBM row-buffer locality in the gather stream. Per 128-point slot the
device does one ACT scale-copy (w0*TL) then a fused DVE multiply-add chain
(3x scalar_tensor_tensor with per-partition f32 weight scalars). 2-slot
gather groups with 6-deep buffering overlap gathers, compute, and the
per-group fp16 output stores; the host converts/unpermutes/unshards.
"""

import math

import numpy as np

B, C, H, W = 8, 256, 128, 128
N_ROIS, NUM_POINT, STRIDE = 2048, 9, 8
NCORES = 8
SG = 2  # slots (of 128 points) per dma_gather call
HW_ = H * W
G_ROWS = HW_ + 2 * W  # doubled-row image height (tail rows zero, never weighted)

_PROGRAM_CACHE: dict[tuple, object] = {}

PAIR_MODE = False
STRIPE = True
# build-time tuning knobs applied by kernel() (overridden by experiments)
BUILD_KWARGS: dict = {"nq": 1, "par_loads": True, "split_idx": True,
                      "gbufs": 6, "wbufs": 3, "compute": "stt"}


def _groups(S, plan=None):
    """Split S slots into gather groups (plan = explicit sizes, else ~SG)."""
    out = []
    s0 = 0
    if plan is not None:
        assert sum(plan) == S, (plan, S)
        for sg in plan:
            out.append((s0, sg))
            s0 += sg
        return out
    while s0 < S:
        sg = min(SG, S - s0)
        out.append((s0, sg))
        s0 += sg
    return out


def _build_program(S: int, iters: int = 1, loop_n: int = 1, plan=None,
                   nq: int = 1, par_loads: bool = False, split_idx: bool = False,
                   gbufs: int = 2, n_img: int = 1, wbufs: int = 2,
                   compute: str = "mt"):
    import concourse.bacc as bacc
    import concourse.mybir as mybir
    import concourse.tile as tile
    from concourse.bass_types import AP

    f32 = mybir.dt.float32
    f16 = mybir.dt.float16
    i16 = mybir.dt.int16
    op = mybir.AluOpType
    C2, C4 = 2 * C, 4 * C

    nc = bacc.Bacc("TRN2", target_bir_lowering=False, debug=False, num_devices=NCORES,
                   num_swdge_queues=nq)
    fm_rows = HW_ * n_img + 2 * W  # images packed at stride HW_; zero tail pad
    fm_t = nc.dram_tensor("fm", [fm_rows, C2], f16, kind="ExternalInput")
    idx_t = nc.dram_tensor("idx", [128, 8 * S], i16, kind="ExternalInput")
    wts_t = nc.dram_tensor("wts", [128, S * 4], f32, kind="ExternalInput")
    out_t = nc.dram_tensor("out", [128, S * C], f16, kind="ExternalOutput")

    # overlapping view: element r = rows [r, r+1] of g = the 4 corners of cell r
    fm_gather_ap = AP(fm_t, 0, [[C2, HW_ * n_img], [1, C4]])

    with tile.TileContext(nc) as tc:
        with (
            tc.tile_pool(name="const", bufs=1) as cpool,
            tc.tile_pool(name="gath", bufs=gbufs) as gpool,
            tc.tile_pool(name="work", bufs=wbufs) as wpool,
        ):
            groups = _groups(S, plan)
            sgmax = max(sg for _, sg in groups)

            idx = cpool.tile([128, 8 * S], i16)
            if split_idx:
                for s0, sg in groups:
                    nc.sync.dma_start(idx[:, s0 * 8 : (s0 + sg) * 8],
                                      idx_t[:, s0 * 8 : (s0 + sg) * 8])
            else:
                nc.sync.dma_start(idx[:], idx_t[:])
            wt = cpool.tile([128, S, 4], f32)
            (nc.scalar if par_loads else nc.sync).dma_start(wt[:], wts_t[:])

            def body():
                for gi, (s0, sg) in enumerate(groups):
                    ttf = gpool.tile([128, sgmax, C4], f16, tag="tt")
                    tt = ttf[:, 0:sg]
                    nc.gpsimd.dma_gather(
                        tt, fm_gather_ap, idx[:, s0 * 8 : (s0 + sg) * 8],
                        sg * 128, sg * 128, C4, elem_step=C2,
                        queue_num=gi % nq,
                    )
                    otf = wpool.tile([128, sgmax, C], f16, tag="ot")
                    ot = otf[:, 0:sg]
                    if compute == "stt":
                        # corners [TL, BL, TR, BR]: ACT seeds w0*TL, then a
                        # fused multiply-add chain on DVE.
                        af = wpool.tile([128, sgmax, 3, C], f16, tag="ac")
                        for sl in range(sg):
                            s = s0 + sl
                            nc.scalar.activation(
                                af[:, sl, 0, :], tt[:, sl, 0:C],
                                mybir.ActivationFunctionType.Copy,
                                bias=0.0, scale=wt[:, s, 0:1],
                            )
                            nc.vector.scalar_tensor_tensor(
                                af[:, sl, 1, :], tt[:, sl, C:C2],
                                wt[:, s, 1:2], af[:, sl, 0, :], op.mult, op.add)
                            nc.vector.scalar_tensor_tensor(
                                af[:, sl, 2, :], tt[:, sl, C2 : 3 * C],
                                wt[:, s, 2:3], af[:, sl, 1, :], op.mult, op.add)
                            nc.vector.scalar_tensor_tensor(
                                ot[:, sl, :], tt[:, sl, 3 * C : C4],
                                wt[:, s, 3:4], af[:, sl, 2, :], op.mult, op.add)
                    else:
                        mtf = wpool.tile([128, sgmax, 4, C], f16, tag="mt")
                        mt = mtf[:, 0:sg]
                        for sl in range(sg):
                            s = s0 + sl
                            # corners: [TL, BL, TR, BR] * weights [w0..w3]
                            nc.scalar.activation(
                                mt[:, sl, 0, :], tt[:, sl, 0:C],
                                mybir.ActivationFunctionType.Copy,
                                bias=0.0, scale=wt[:, s, 0:1],
                            )
                            nc.vector.tensor_scalar(
                                mt[:, sl, 1, :], tt[:, sl, C:C2],
                                wt[:, s, 1:2], None, op.mult,
                            )
                            nc.vector.tensor_scalar(
                                mt[:, sl, 2, :], tt[:, sl, C2 : 3 * C],
                                wt[:, s, 2:3], None, op.mult,
                            )
                            nc.vector.tensor_scalar(
                                mt[:, sl, 3, :], tt[:, sl, 3 * C : C4],
                                wt[:, s, 3:4], None, op.mult,
                            )
                        mv = mt.rearrange("p s (a b) c -> p s a b c", a=2, b=2)
                        pvf = wpool.tile([128, sgmax, 2, C], f16, tag="pv")
                        pv = pvf[:, 0:sg]
                        nc.vector.tensor_tensor(
                            pv, mv[:, :, :, 0, :], mv[:, :, :, 1, :], op.add
                        )
                        nc.vector.tensor_tensor(
                            ot, pv[:, :, 0, :], pv[:, :, 1, :], op.add
                        )
                    nc.sync.dma_start(
                        out_t[:, s0 * C : (s0 + sg) * C],
                        ot.rearrange("p s c -> p (s c)"),
                    )

            if loop_n > 1:
                with tc.For_i(0, loop_n):
                    for _it in range(iters):
                        body()
            else:
                for _it in range(iters):
                    body()

    nc.compile()
    return nc


def _get_program(S: int, n_img: int = 1):
    key = (S, n_img)
    if key not in _PROGRAM_CACHE:
        _PROGRAM_CACHE[key] = _build_program(S, n_img=n_img, **BUILD_KWARGS)
    return _PROGRAM_CACHE[key]


def _point_fields(rois, offset, num_point):
    """Per-point gather index + 4 bilinear corner weights (reference math)."""
    n = rois.shape[0]
    cx = (rois[:, 1] + rois[:, 3]) * np.float32(0.5)
    cy = (rois[:, 2] + rois[:, 4]) * np.float32(0.5)
    w_ = rois[:, 3] - rois[:, 1] + np.float32(1.0)
    h_ = rois[:, 4] - rois[:, 2] + np.float32(1.0)
    off = offset.reshape(n, num_point, 2)
    inv_s = np.float32(1.0 / STRIDE)
    x = (cx[:, None] + off[:, :, 0] * (w_[:, None] * np.float32(0.1))) * inv_s
    y = (cy[:, None] + off[:, :, 1] * (h_[:, None] * np.float32(0.1))) * inv_s

    xl = np.clip(np.floor(x), 0.0, W - 1).astype(np.float32)
    yl = np.clip(np.floor(y), 0.0, H - 1).astype(np.float32)
    lw = np.where(xl >= W - 1, np.float32(0.0), x - xl).astype(np.float32)
    lh = np.where(yl >= H - 1, np.float32(0.0), y - yl).astype(np.float32)
    cw = np.float32(1.0) - lw
    ch = np.float32(1.0) - lh

    idx = (yl.astype(np.int32) * W + xl.astype(np.int32)).astype(np.int16)
    wts = np.stack([ch * cw, lh * cw, ch * lw, lh * lw], axis=-1)  # [n,P,4]
    return idx.reshape(-1), wts.reshape(-1, 4).astype(np.float32)


def _stripe_major(NP):
    """Position j takes sequence entry (j%16)*(NP//16) + j//16, giving each
    16-partition idx stripe a contiguous block of the (sorted) sequence."""
    j = np.arange(NP)
    return (j % 16) * (NP // 16) + (j // 16)


def _host_prep(feat_map, rois, offset, num_point, sort=True, stripe=False):
    """Route rois by batch index; build per-core device inputs.

    sort=True permutes each core's points into ascending gather-cell order
    (HBM row-buffer locality); the inverse permutation is applied on unshard.
    """
    bidx = rois[:, 0].astype(np.int32)
    ids = [np.nonzero(bidx == b)[0] for b in range(B)]
    cap = max(len(i) for i in ids)
    S = math.ceil(max(cap * num_point, 1) / 128)
    NP = S * 128

    in_maps = []
    perms = []
    for b in range(B):
        fmb = feat_map[b].transpose(1, 2, 0).reshape(HW_, C).astype(np.float16)
        g = np.zeros((G_ROWS, 2 * C), np.float16)
        g[:HW_, :C] = fmb
        g[: HW_ - W, C:] = fmb[W:]
        idl = ids[b]
        nb = len(idl)
        idx_flat = np.zeros(NP, np.int16)
        wts_flat = np.zeros((NP, 4), np.float32)
        pos = None  # final device row of each original point
        if nb:
            npts = nb * num_point
            pi, pw = _point_fields(rois[idl], offset[idl], num_point)
            owner = np.full(NP, -1, np.int64)  # device row -> original point
            owner[:npts] = np.arange(npts)
            if sort:
                perm = np.argsort(pi, kind="stable")
                pi = pi[perm]
                pw = pw[perm]
                owner[:npts] = perm
            idx_flat[:npts] = pi
            wts_flat[:npts] = pw
            if stripe:
                sm = _stripe_major(NP)
                idx_flat = idx_flat[sm]
                wts_flat = wts_flat[sm]
                owner = owner[sm]
            pos = np.empty(npts, np.int64)
            valid = owner >= 0
            pos[owner[valid]] = np.nonzero(valid)[0]
        perms.append(pos)
        band = np.ascontiguousarray(idx_flat.reshape(8 * S, 16).T)  # [16, 8S]
        idx128 = np.tile(band, (8, 1))
        wts = np.ascontiguousarray(
            wts_flat.reshape(S, 128, 4).transpose(1, 0, 2)
        ).reshape(128, S * 4)
        in_maps.append({"fm": g, "idx": idx128, "wts": wts})
    return ids, S, in_maps, perms


def _host_unshard(results, ids, S, num_point, n, perms=None):
    out_full = np.zeros((n, num_point, C), np.float32)
    for b in range(B):
        nb = len(ids[b])
        if not nb:
            continue
        o = results[b]["out"].astype(np.float32)
        o = o.reshape(128, S, C).transpose(1, 0, 2).reshape(S * 128, C)
        if perms is not None and perms[b] is not None:
            o = o[perms[b]]
        else:
            o = o[: nb * num_point]
        out_full[ids[b]] = o.reshape(nb, num_point, C)
    return out_full


def _host_prep_pair(feat_map, rois, offset, num_point, stripe=False):
    """Pack 2 images per core (idx stride HW_); split each pair's points
    across 2 cores in sorted-cell order. Returns (S, in_maps, gpis)."""
    n = rois.shape[0]
    bidx = rois[:, 0].astype(np.int32)
    ids = [np.nonzero(bidx == b)[0] for b in range(B)]
    pi_all, pw_all = _point_fields(rois, offset, num_point)
    pi_all = pi_all.astype(np.int32).reshape(n, num_point)
    pw_all = pw_all.reshape(n, num_point, 4)

    halves = {}  # core -> (cells, wts, gpis)
    fms = {}
    npair = B // 2
    for p in range(npair):
        cells, wts, gpis = [], [], []
        for k in (0, 1):
            idl = ids[2 * p + k]
            cells.append(pi_all[idl].reshape(-1) + k * HW_)
            wts.append(pw_all[idl].reshape(-1, 4))
            gpis.append((idl[:, None] * num_point +
                         np.arange(num_point)[None, :]).reshape(-1))
        cells = np.concatenate(cells)
        wts = np.concatenate(wts)
        gpis = np.concatenate(gpis)
        order = np.argsort(cells, kind="stable")
        half = (len(order) + 1) // 2
        for k, sel in enumerate((order[:half], order[half:])):
            halves[p + 4 * k] = (cells[sel], wts[sel], gpis[sel])
        g = np.zeros((HW_ * 2 + 2 * W, 2 * C), np.float16)
        for k in (0, 1):
            fmb = feat_map[2 * p + k].transpose(1, 2, 0).reshape(HW_, C)
            fmb = fmb.astype(np.float16)
            g[k * HW_ : (k + 1) * HW_, :C] = fmb
            g[k * HW_ : (k + 1) * HW_ - W, C:] = fmb[W:]
        fms[p] = g

    cap = max(len(v[0]) for v in halves.values())
    S = math.ceil(max(cap, 1) / 128)
    NP = S * 128
    in_maps, gpis_out = [], []
    for c in range(NCORES):
        cells, wts, gpis = halves[c]
        m = len(cells)
        idx_flat = np.zeros(NP, np.int16)
        wts_flat = np.zeros((NP, 4), np.float32)
        idx_flat[:m] = cells.astype(np.int16)
        wts_flat[:m] = wts
        rows = np.arange(m)
        if stripe and m:
            sm = _stripe_major(NP)
            idx_flat = idx_flat[sm]
            wts_flat = wts_flat[sm]
            owner = np.full(NP, -1, np.int64)
            owner[:m] = np.arange(m)
            owner = owner[sm]
            rows = np.empty(m, np.int64)
            valid = owner >= 0
            rows[owner[valid]] = np.nonzero(valid)[0]
        band = np.ascontiguousarray(idx_flat.reshape(8 * S, 16).T)
        idx128 = np.tile(band, (8, 1))
        w128 = np.ascontiguousarray(
            wts_flat.reshape(S, 128, 4).transpose(1, 0, 2)
        ).reshape(128, S * 4)
        in_maps.append({"fm": fms[c % 4], "idx": idx128, "wts": w128})
        gpis_out.append((rows, gpis))
    return S, in_maps, gpis_out


def _host_unshard_pair(results, S, gpis, n, num_point):
    out_flat = np.zeros((n * num_point, C), np.float32)
    for c in range(NCORES):
        o = results[c]["out"].astype(np.float32)
        o = o.reshape(128, S, C).transpose(1, 0, 2).reshape(S * 128, C)
        rows, gpi = gpis[c]
        out_flat[gpi] = o[rows]
    return out_flat.reshape(n, num_point, C)


def kernel(feat_map, rois, offset, stride, num_point, _collect=None):
    from concourse.bass_utils import run_bass_kernel_spmd

    feat_map = np.ascontiguousarray(np.asarray(feat_map, np.float32))
    rois = np.asarray(rois, np.float32)
    offset = np.asarray(offset, np.float32)
    stride = int(stride)
    num_point = int(num_point)
    assert feat_map.shape == (B, C, H, W), feat_map.shape
    assert stride == STRIDE and num_point == NUM_POINT

    if PAIR_MODE:
        S, in_maps, gpis = _host_prep_pair(feat_map, rois, offset, num_point,
                                           stripe=STRIPE)
        nc = _get_program(S, n_img=2)
    else:
        ids, S, in_maps, perms = _host_prep(feat_map, rois, offset, num_point,
                                            stripe=STRIPE)
        nc = _get_program(S)
    res = run_bass_kernel_spmd(nc, in_maps, core_ids=list(range(NCORES)),
                               **(_collect.pop("spmd_kwargs", {}) if _collect else {}))
    if _collect is not None:
        _collect["res"] = res
    if PAIR_MODE:
        return _host_unshard_pair(res.results, S, gpis, rois.shape[0], num_point)
    return _host_unshard(res.results, ids, S, num_point, rois.shape[0], perms)
